# revision 12
# baseline (speedup 1.0000x reference)
"""Trainium2 Bass kernel for nn_LocalMaxSA (B=8, T=2048, F=256).

Per batch b (one NeuronCore each):
    q = feat @ Wq^T + bq;  k = feat @ Wk^T + bk
    sa = q @ k^T                       [T, T]
    score = mean(sa, -1) = q @ (sum_s k[s]) / T
    attn = softmax(sa, -1)   (no max subtraction needed: |sa| <~ 40)
    feat_new = attn @ feat
    hlens_new = T (constant)

Device layout (all matmul operands float32r for the 1-cycle/row PE path):
    feat_sb [128, 16, 257]  : feat tiles [s' part, j, f] with a ones column
    featT   [128, 2, 2048]  : feat transposed (f on partitions)  via TensorE
    qT/kT   [128, 2, 2048]  : projections, g on partitions (2 tiles)
    sT tiles [s'=128, t=512] in PSUM -> Exp on ScalarE -> expT [128,16,512]
    feat_new[t,f] + denom: matmul(lhsT=expT block, rhs=feat_sb[.., 257]),
    accumulate 16 s'-tiles; divide by the ones-column sum; DMA out.
"""
import numpy as np

import concourse.bacc as bacc
import concourse.mybir as mybir
import concourse.tile as tile
from concourse.bass_utils import run_bass_kernel_spmd
from concourse.masks import make_identity

B, T, F = 8, 2048, 256
JT = T // 128          # 16 row tiles
NCH = 4                # t-chunks of 512
CH = T // NCH
F32 = mybir.dt.float32
F32R = mybir.dt.float32r
AF = mybir.ActivationFunctionType

_NC_CACHE = {}


def _body(nc, tc, feat_d, wqt_d, wkt_d, bq_d, bk_d, ofeat_d, oscore_d, ctx):
    pp = ctx.enter_context(tc.tile_pool(name="persist", bufs=1))
    feat_sb = pp.tile([128, JT, F + 2], F32R, tag="feat")
    featT = pp.tile([128, 2, T], F32R, tag="featT")
    qT = pp.tile([128, 2, T], F32R, tag="qT")
    kT = pp.tile([128, 2, T], F32R, tag="kT")
    wqT = pp.tile([128, 2, F], F32R, tag="wqT")
    wkT = pp.tile([128, 2, F], F32R, tag="wkT")
    bq_sb = pp.tile([128, 2], F32, tag="bq")
    bk_sb = pp.tile([128, 2], F32, tag="bk")
    ident = pp.tile([128, 128], F32R, tag="ident")
    ident32 = pp.tile([128, 128], F32, tag="ident32")
    ones32 = pp.tile([128, JT, 2], F32, tag="ones32")
    ksum = pp.tile([128, 2], F32, tag="ksum")
    score_sb = pp.tile([128, JT], F32, tag="score")

    make_identity(nc, ident32[:])
    nc.vector.tensor_copy(ident[:], ident32[:])
    nc.vector.memset(ones32[:], 1.0)
    nc.vector.tensor_copy(feat_sb[:, :, F : F + 2], ones32[:])
    feat_t = feat_d[:].bitcast(F32R).rearrange("(j p) f -> j p f", p=128)
    for j in range(JT):
        eng = nc.sync if j % 2 == 0 else nc.gpsimd
        eng.dma_start(feat_sb[:, j, 0:F], feat_t[j])
    nc.sync.dma_start(
        wqT[:], wqt_d[:].bitcast(F32R).rearrange("(h p) g -> p h g", p=128)
    )
    nc.sync.dma_start(
        wkT[:], wkt_d[:].bitcast(F32R).rearrange("(h p) g -> p h g", p=128)
    )
    nc.sync.dma_start(bq_sb[:], bq_d[:].rearrange("h p -> p h"))
    nc.sync.dma_start(bk_sb[:], bk_d[:].rearrange("h p -> p h"))

    with (
        tc.tile_pool(name="ps_tp", bufs=4, space="PSUM") as ps_tp,
        tc.tile_pool(name="ps_pr", bufs=3, space="PSUM") as ps_pr,
        tc.tile_pool(name="ps_scp", bufs=1, space="PSUM") as ps_scp,
    ):
        def proj(wT, b_sb, outT, m, c):
            pq = ps_pr.tile([128, CH], F32, tag="pr")
            for h in range(2):
                nc.tensor.matmul(
                    pq[:],
                    wT[:, h, m * 128 : (m + 1) * 128],
                    featT[:, h, c * CH : (c + 1) * CH],
                    start=(h == 0),
                    stop=(h == 1),
                )
            nc.vector.tensor_scalar_add(
                outT[:, m, c * CH : (c + 1) * CH], pq[:], b_sb[:, m : m + 1]
            )

        # feat^T via TensorE (fp32 has no DMA-transpose path), interleaved with
        # the k projection of each finished t-chunk so PE never waits on the
        # PSUM->SBUF evacuation copies. k before q: sT needs all of kT but
        # only one qT chunk.
        for c in range(NCH):
            for j in range(4 * c, 4 * c + 4, 2):
                for h in range(2):
                    pt = ps_tp.tile([128, 2, 128], F32R, tag="tp")
                    for jj in range(2):
                        nc.tensor.transpose(
                            pt[:, jj, :],
                            feat_sb[:, j + jj, h * 128 : (h + 1) * 128],
                            ident[:],
                        )
                    nc.any.tensor_copy(
                        featT[:, h, j * 128 : (j + 2) * 128], pt[:]
                    )
            for m in range(2):
                proj(wkT, bk_sb, kT, m, c)
        for c in range(NCH):
            for m in range(2):
                proj(wqT, bq_sb, qT, m, c)

        # score = q . ksum / T  (tiny; PE does it here while ACT/DVE are idle,
        # its DVE/ACT/DMA tail overlaps the main loop)
        for m in range(2):
            nc.vector.reduce_sum(
                ksum[:, m : m + 1], kT[:, m, :], axis=mybir.AxisListType.X
            )
        psc = ps_scp.tile([128, JT], F32, tag="sc")
        for i in range(JT):
            for m in range(2):
                nc.tensor.matmul(
                    psc[:, i : i + 1],
                    qT[:, m, i * 128 : (i + 1) * 128].bitcast(F32),
                    ksum[:, m : m + 1],
                    start=(m == 0),
                    stop=(m == 1),
                )
        nc.scalar.mul(score_sb[:], psc[:], 1.0 / T)
        nc.sync.dma_start(oscore_d[:].rearrange("j p -> p j"), score_sb[:])

    ofeat_t = ofeat_d[:].rearrange("(i p) f -> i p f", p=128)
    with (
        tc.tile_pool(name="ps_s", bufs=3, space="PSUM") as ps_s,
        tc.tile_pool(name="ps_f", bufs=2, space="PSUM") as ps_f,
        tc.tile_pool(name="work", bufs=2) as wk,
    ):
        for c in range(NCH):
            expT = wk.tile([128, JT, CH], F32R, tag="expT")
            for j in range(0, JT, 2):
                ps = ps_s.tile([128, 2, CH], F32, tag="s")
                for jj in range(2):
                    for m in range(2):
                        nc.tensor.matmul(
                            ps[:, jj, :],
                            kT[:, m, (j + jj) * 128 : (j + jj + 1) * 128],
                            qT[:, m, c * CH : (c + 1) * CH],
                            start=(m == 0),
                            stop=(m == 1),
                        )
                nc.scalar.activation(expT[:, j : j + 2, :], ps[:], AF.Exp)
            for u in range(CH // 128):
                i = (c * CH) // 128 + u
                pf = ps_f.tile([128, F + 2], F32, tag="f")
                for j in range(JT):
                    nc.tensor.matmul(
                        pf[:],
                        expT[:, j, u * 128 : (u + 1) * 128],
                        feat_sb[:, j, :],
                        start=(j == 0),
                        stop=(j == JT - 1),
                    )
                rec = wk.tile([128, 1], F32, tag="rec")
                nc.vector.reciprocal(rec[:], pf[:, F : F + 1])
                fnew = wk.tile([128, F], F32, tag="fnew")
                nc.vector.tensor_scalar_mul(fnew[:], pf[:, 0:F], rec[:])
                nc.sync.dma_start(ofeat_t[i], fnew[:])



def _build():
    nc = bacc.Bacc("TRN2", target_bir_lowering=False, debug=False, num_devices=B)
    feat_d = nc.dram_tensor("feat", [T, F], F32, kind="ExternalInput")
    wqt_d = nc.dram_tensor("wqt", [F, F], F32, kind="ExternalInput")
    wkt_d = nc.dram_tensor("wkt", [F, F], F32, kind="ExternalInput")
    bq_d = nc.dram_tensor("bq", [2, 128], F32, kind="ExternalInput")
    bk_d = nc.dram_tensor("bk", [2, 128], F32, kind="ExternalInput")
    ofeat_d = nc.dram_tensor("ofeat", [T, F], F32, kind="ExternalOutput")
    oscore_d = nc.dram_tensor("oscore", [JT, 128], F32, kind="ExternalOutput")
    from contextlib import ExitStack

    with tile.TileContext(nc) as tc, ExitStack() as ctx:
        _body(nc, tc, feat_d, wqt_d, wkt_d, bq_d, bk_d, ofeat_d, oscore_d, ctx)
    nc.compile()
    return nc


def get_nc():
    if "nc" not in _NC_CACHE:
        _NC_CACHE["nc"] = _build()
    return _NC_CACHE["nc"]


def run(in_maps, **kw):
    return run_bass_kernel_spmd(get_nc(), in_maps, list(range(B)), **kw)


def make_in_maps(feat, Wq, bq, Wk, bk):
    wqt = np.ascontiguousarray(np.asarray(Wq, np.float32).T)
    wkt = np.ascontiguousarray(np.asarray(Wk, np.float32).T)
    bq2 = np.ascontiguousarray(np.asarray(bq, np.float32).reshape(2, 128))
    bk2 = np.ascontiguousarray(np.asarray(bk, np.float32).reshape(2, 128))
    return [
        {
            "feat": np.ascontiguousarray(np.asarray(feat[c], np.float32)),
            "wqt": wqt,
            "wkt": wkt,
            "bq": bq2,
            "bk": bk2,
        }
        for c in range(B)
    ]


def kernel(feat, hlens, Wq, bq, Wk, bk):
    res = run(make_in_maps(feat, Wq, bq, Wk, bk))
    feat_new = np.stack([res.results[c]["ofeat"] for c in range(B)])
    score = np.stack([res.results[c]["oscore"].reshape(T) for c in range(B)])
    hlens_new = np.full((B,), T, dtype=np.int32)
    return feat_new, hlens_new, score
